# revision 32
# baseline (speedup 1.0000x reference)
"""Trainium2 Bass kernel for nn_BasicBlock (WeightNet/CondConv-style block).

Data parallel over batch: 32 samples -> 8 cores x 4 samples.

Conv: 1D Winograd F(2,3) along W (M-form). Even/odd column planes
(host-prepadded) -> 4 V planes per row-strip via DVE/Pool adds.

Weight generation runs in W^T orientation ([oc partitions, (cc,kw,kh,cin)
free]) where the rank-4 mixing coefficients are *per-partition* f32
scalars: 4x tensor_scalar_mul (4x DVE mode) + 3 adds replaces the old
broadcast-tile + 7 tensor_tensor passes, and the aexp broadcast matmuls
disappear.  The 1D Winograd U-transform runs as slab ops (uu/u1/u2) in
W^T, then four dma_start_transpose ops per (occ,cc) drop the stationary
tiles [cin, 12, oc] directly (kw0 | u1 | u2 | kw2 slabs).  BN scale and
the halving of the centre tap are folded into the host-packed basis.

The Y-stage runs as: one ACT evacuation of the 4 M psum planes, two
merged 2-lane DVE ops, one merged relu(+bias) ACT into the next conv's
input planes (U2 slab is built negated so eo2 = eo1 - m[2:4] works).
"""

import sys

sys.path.insert(0, "/opt/trn_rl_repo")

import numpy as np
import ml_dtypes

import concourse.bass as bass
import concourse.tile as tile
from concourse import bacc, mybir
from concourse import bass_utils

F32 = mybir.dt.float32
BF16 = mybir.dt.bfloat16
AF = mybir.ActivationFunctionType
ALU = mybir.AluOpType

B, C, H, W = 32, 256, 56, 56
NCORES = 8
BL = B // NCORES          # samples per core
RP = H + 2                # padded rows: 58
TC = W // 2               # tile cols: 28
PW = TC + 2               # plane width: 30
NT, TR = 7, 8             # row-groups
NG = TR * TC              # 224 cols per M plane slice
EPS = 1e-5


def build_program():
    nc = bacc.Bacc("TRN2", target_bir_lowering=False, debug=False,
                   num_devices=NCORES)

    # host-prepadded planes: [s, cc, 128, (o-plane, e-plane), RP, PW]
    xeo4 = nc.dram_tensor("xeo4", [BL, 2, 128, 2, RP, PW], BF16,
                          kind="ExternalInput").ap()
    # out: [s, occ, 128, (even-cols, odd-cols), H, TC]
    outd = nc.dram_tensor("outd", [BL, 2, 128, 2, H, TC], BF16,
                          kind="ExternalOutput").ap()
    rwT = nc.dram_tensor("rwT", [2, 128, 16], F32, kind="ExternalInput").ap()
    rb = nc.dram_tensor("rb", [16, 1], F32, kind="ExternalInput").ap()
    fc1wT = [nc.dram_tensor(f"fc1wT{n}", [16, 4096], BF16,
                            kind="ExternalInput").ap() for n in (1, 2)]
    fc1b = [nc.dram_tensor(f"fc1b{n}", [128, 32], F32,
                           kind="ExternalInput").ap() for n in (1, 2)]
    # W^T basis: [i, occ, 128 oc, (cc 2, kw 3, kh 3, cin 128)]
    basd = [nc.dram_tensor(f"bas{n}", [4, 2, 128, 2304], BF16,
                           kind="ExternalInput").ap() for n in (1, 2)]
    bnb = [nc.dram_tensor(f"bnb{n}", [2, 128, 1], F32,
                          kind="ExternalInput").ap() for n in (1, 2)]

    with tile.TileContext(nc) as tc:
        build_body(tc, xeo4, outd, rwT, rb, fc1wT, fc1b, basd, bnb)

    nc.compile()
    return nc


def build_body(tc, xeo4, outd, rwT, rb, fc1wT, fc1b, basd, bnb):
    nc = tc.nc
    from contextlib import ExitStack
    ctx = ExitStack()

    cpool = ctx.enter_context(tc.tile_pool(name="consts", bufs=1))
    xeo_p = ctx.enter_context(tc.tile_pool(name="xeo", bufs=2))
    twS_p = ctx.enter_context(tc.tile_pool(name="twS", bufs=6))
    wt9_p = ctx.enter_context(tc.tile_pool(name="wt9", bufs=1))
    wst_p = ctx.enter_context(tc.tile_pool(name="wst", bufs=2))
    usl_p = ctx.enter_context(tc.tile_pool(name="uslab", bufs=2))
    ctmp_p = ctx.enter_context(tc.tile_pool(name="ctmp", bufs=2))
    small_p = ctx.enter_context(tc.tile_pool(name="small", bufs=2))
    stage_p = ctx.enter_context(tc.tile_pool(name="stage", bufs=2))
    psum_p = ctx.enter_context(tc.tile_pool(name="psum", bufs=3, space="PSUM"))
    psmall_p = ctx.enter_context(tc.tile_pool(name="psmall", bufs=1,
                                              space="PSUM"))
    dram_p = ctx.enter_context(tc.tile_pool(name="dscratch", bufs=2,
                                            space="DRAM"))

    # ---- resident constants ----
    # basis tiles: [wn][i][occ] -> [128 oc, 2 cc, 9 (kw,kh), 128 cin]
    bas_sb = []
    for n in range(2):
        per = []
        for i in range(4):
            per.append([cpool.tile([128, 2, 9, 128], BF16,
                                   tag=f"bas{n}{i}{o}", name=f"bas{n}{i}{o}")
                        for o in range(2)])
        bas_sb.append(per)

    def load_basis(n, i):
        for o in range(2):
            nc.sync.dma_start(
                bas_sb[n][i][o][:],
                basd[n][i, o].rearrange("p (c k m) -> p c k m", c=2, k=9))

    rwT_sb = []
    for c in range(2):
        t = cpool.tile([128, 16], F32, tag=f"rwT{c}")
        nc.sync.dma_start(t[:], rwT[c])
        rwT_sb.append(t)
    rb_sb = cpool.tile([16, 1], F32, tag="rb")
    nc.sync.dma_start(rb_sb[:], rb)
    fc1b_sb, bnb_sb = [], []
    fc1w_t = cpool.tile([16, 4096], BF16, tag="fc1w")
    for n in range(2):
        t = cpool.tile([128, 32], F32, tag=f"fc1b{n}")
        if n == 0:
            nc.sync.dma_start(t[:], fc1b[n])
        fc1b_sb.append(t)
        tb = [cpool.tile([128, 1], F32, tag=f"bnb{n}{c}", name=f"bnbt{n}{c}")
              for c in range(2)]
        bnb_sb.append(tb)

    def load_deferred_consts():
        nc.sync.dma_start(fc1b_sb[1][:], fc1b[1])
        for n in range(2):
            for c in range(2):
                nc.sync.dma_start(bnb_sb[n][c][:], bnb[n][c])

    gap16 = cpool.tile([16, BL], BF16, tag="gap16")
    garb = cpool.tile([128, RP * PW], BF16, tag="garb")

    # conv1 output planes (= conv2 input planes): [128, (o,e), RP, PW]
    # zeroed once; sinks only ever write rows 1..56, cols 1..28
    yeo = [cpool.tile([128, 2, RP, PW], BF16, tag=f"yeo{c}", name=f"yeo{c}")
           for c in range(2)]
    for c in range(2):
        nc.gpsimd.memset(yeo[c][:], 0.0)

    # PE filler: tiny matmuls that keep the tensor engine "hot" (p-state)
    # during unavoidable PE gaps.  fill_a/fill_b are constants; fill(dep)
    # emits a matmul whose moving operand is a slice of `dep`, pacing it
    # behind that tile's producer.
    fill_a = cpool.tile([128, 2], BF16, tag="fill_a")
    nc.gpsimd.memset(fill_a[:], 0.0)
    fill_ps = psmall_p.tile([128, 512], F32, tag="fill_ps", name="fill_ps")

    def fill(dep_ap, n=1):
        sz = 1
        for d in dep_ap.shape[1:]:
            sz *= d
        for _ in range(n):
            nc.tensor.matmul(fill_ps[0:2, 0:sz], fill_a[:],
                             dep_ap, start=True, stop=True)

    # ---- weight generation ----
    def gen_weights_a_ops(wn, s, ops):
        """sigmoid(fc1(gap)) -> DRAM roundtrip -> per-partition scalar
        tile avt2 [128 oc_lo, occ, (q,i)]."""
        apsg = psmall_p.tile([128, 33], F32, tag="avec_ps")
        aps = apsg[:, 0:32]
        avt = small_p.tile([128, 32], F32, tag="avtmp")
        avecf = small_p.tile([128, 32], F32, tag="avecf")
        avd = dram_p.tile([4096], F32, tag="avd")
        avt2 = small_p.tile([128, 2, 16], F32, tag="avt2")

        ops.append(lambda: nc.sync.dma_start(fc1w_t[:], fc1wT[wn]))

        def avec_mms(j0):
            def f():
                for j in range(j0, j0 + 8):
                    nc.tensor.matmul(aps[:, j:j + 1],
                                     fc1w_t[:, 128 * j:128 * (j + 1)],
                                     gap16[:, s:s + 1],
                                     start=True, stop=True)
            return f
        for j0 in range(0, 32, 8):
            ops.append(avec_mms(j0))
        ops.append(lambda: nc.vector.tensor_add(avt[:], aps, fc1b_sb[wn][:]))
        ops.append(lambda: nc.scalar.activation(avecf[:], avt[:], AF.Sigmoid))
        # roundtrip: a[l], l = oc*16 + q*4 + i  ->  avt2[p, occ, (q i)]
        ops.append(lambda: nc.sync.dma_start(
            avd[:].rearrange("(j p) -> p j", p=128), avecf[:]))
        ops.append(lambda: nc.sync.dma_start(
            avt2[:], avd[:].rearrange("(o p c) -> p o c", o=2, c=16)))
        return avt2

    def gen_weights_b_ops(wn, avt2, ops, pace=False):
        """W^T combine -> U slabs -> dma-transposes -> stationary tiles.

        Returns wst[cc][occ] tiles [128 cin, 12 (wpos,kh), 128 oc]."""
        wt9 = [wt9_p.tile([128, 2, 9, 128], BF16, tag=f"wt9{o}",
                          name=f"wt9_{o}") for o in range(2)]
        wst = [[wst_p.tile([128, 12, 128], BF16, tag=f"wst{c}{o}",
                           name=f"wst{c}{o}") for o in range(2)]
               for c in range(2)]

        # combine: wt9[occ][:, cc, :, q*64:...] = sum_i bas_i * a_i
        def combine(o, c, qh):
            q = 2 * c + qh
            sl = (slice(None), c, slice(None), slice(64 * qh, 64 * qh + 64))

            def sca(i):
                return avt2[:, o, 4 * q + i:4 * q + i + 1]

            tm = [ctmp_p.tile([128, 9, 64], BF16, tag=f"ctm{k % 2}",
                              name=f"ctm{k}") for k in range(3)]

            def f1():
                acc = wt9[o][sl]
                nc.vector.tensor_scalar_mul(acc, bas_sb[wn][0][o][sl], sca(0))
                for i in range(1, 4):
                    nc.vector.tensor_scalar_mul(tm[i - 1][:],
                                                bas_sb[wn][i][o][sl], sca(i))

            def f2():
                acc = wt9[o][sl]
                nc.vector.tensor_add(acc, acc, tm[0][:])
                nc.vector.tensor_add(acc, acc, tm[1][:])
                nc.vector.tensor_add(acc, acc, tm[2][:])
                if pace:
                    fill(wt9[o][:, c, 0:4, 64 * qh:64 * qh + 64], 2)
            return f1, f2
        # U slabs + transposes:  kw0 | u1 | u2(neg) | kw2
        #   uuh = 0.5*(W0+W2);  u1 = uuh + W1h ;  u2 = W1h - uuh
        # (W1h pre-halved on host.)
        def mk_uops_tps(o, c):
            uu = usl_p.tile([128, 3, 128], BF16, tag="uu", bufs=1,
                            name=f"uu{c}{o}")
            uuh = usl_p.tile([128, 3, 128], BF16, tag="uuh", bufs=1,
                             name=f"uuh{c}{o}")
            u1s = usl_p.tile([128, 3, 128], BF16, tag="u1s",
                             name=f"u1s{c}{o}")
            u2s = usl_p.tile([128, 3, 128], BF16, tag="u2s",
                             name=f"u2s{c}{o}")
            kw0 = wt9[o][:, c, 0:3, :]
            kw1 = wt9[o][:, c, 3:6, :]
            kw2 = wt9[o][:, c, 6:9, :]

            def uops():
                nc.gpsimd.tensor_add(uu[:], kw0, kw2)
                nc.vector.tensor_scalar_mul(uuh[:], uu[:], 0.5)
                nc.gpsimd.tensor_add(u1s[:], uuh[:], kw1)
                nc.gpsimd.tensor_sub(u2s[:], kw1, uuh[:])
                if pace:
                    fill(u1s[:, :, 0:128], 2)
                    fill(u2s[:, :, 0:128], 2)

            def tps():
                w = wst[c][o]
                nc.sync.dma_start_transpose(w[:, 0:3, :], kw0)
                nc.sync.dma_start_transpose(w[:, 3:6, :], u1s[:])
                nc.sync.dma_start_transpose(w[:, 6:9, :], u2s[:])
                nc.sync.dma_start_transpose(w[:, 9:12, :], kw2)
            return uops, tps

        for o in range(2):
            for c in range(2):
                f1a, f2a = combine(o, c, 0)
                f1b, f2b = combine(o, c, 1)
                uops, tps = mk_uops_tps(o, c)
                ops.extend([f1a, f2a, f1b, f2b, uops, tps])
        return wst

    def stat(wu, cc, kh, wpos, occ):
        return wu[cc][occ][:, 3 * wpos + kh, :]

    # ---- x loading + gap ----
    def load_x_ops(s, ops):
        planes = []
        for c in range(2):
            te = xeo_p.tile([128, 2, RP, PW], BF16, tag=f"xeo{c}",
                            name=f"xeo{c}")
            planes.append(te)
        gacc = []
        for c in range(2):
            ga = [small_p.tile([128, 1], F32, tag=f"gacc{c}{a}",
                               name=f"gacc{c}{a}") for a in range(2)]
            gs = small_p.tile([128, 1], F32, tag=f"gsum{c}", name=f"gsum{c}")
            gacc.append((ga, gs))
        apsg = psmall_p.tile([128, 33], F32, tag="avec_ps", name="gapps")
        gps = apsg[0:16, 32:33]

        for c in range(2):
            ops.append(lambda c=c: nc.sync.dma_start(planes[c][:], xeo4[s, c]))
        for c in range(2):
            for a in range(2):
                ops.append(lambda c=c, a=a: nc.scalar.activation(
                    garb[:], planes[c][:, a].rearrange("p h w -> p (h w)"),
                    AF.Copy, accum_out=gacc[c][0][a][:]))
            ops.append(lambda c=c: nc.vector.tensor_add(
                gacc[c][1][:], gacc[c][0][0][:], gacc[c][0][1][:]))

        def gapmm():
            for c in range(2):
                nc.tensor.matmul(gps, rwT_sb[c][:], gacc[c][1][:],
                                 start=(c == 0), stop=(c == 1))
            nc.scalar.activation(gap16[:, s:s + 1], gps, AF.Identity,
                                 bias=rb_sb[:], scale=1.0)
        ops.append(gapmm)
        return planes

    # ---- conv: per-t-group V strips + matmuls + sink ----
    # V planes (from o-plane ol/orr, e-plane el/er):
    #   V0 = el - er; V1 = ol + er; V2 = er - ol; V3 = ol - orr
    def buildV(planes, t):
        r0 = TR * t
        tws = []
        for c in range(2):
            st = twS_p.tile([128, 4, TR + 2, TC], BF16, tag=f"tws{c}",
                            name=f"tws{c}")
            e = planes[c][:, 1, r0:r0 + TR + 2, :]
            o = planes[c][:, 0, r0:r0 + TR + 2, :]
            el = e[:, :, 0:TC]
            er = e[:, :, 1:TC + 1]
            ol = o[:, :, 1:TC + 1]
            orr = o[:, :, 2:TC + 2]
            nc.vector.tensor_sub(st[:, 0], el, er)
            nc.vector.tensor_add(st[:, 1], ol, er)
            nc.gpsimd.tensor_sub(st[:, 2], er, ol)
            nc.gpsimd.tensor_sub(st[:, 3], ol, orr)
            tws.append(st)
        return tws

    def conv(wu, planes, sink, sched=None, pre=None):
        """V strips are built two groups ahead (before the current group's
        ystage ops, so DVE/Pool chew them while PE runs group t).
        sched[t] is a list of background thunks emitted at group t.
        pre = [tws(0), tws(1)] built by the previous conv's stream."""
        if pre is None:
            pre = [buildV(planes, 0), buildV(planes, 1)]
        pipe = list(pre)
        for t in range(NT):
            if t + 2 < NT:
                pipe.append(buildV(planes, t + 2))
            if sched:
                for f in sched[t]:
                    f()
            tws = pipe.pop(0)
            for occ in range(2):
                ps = psum_p.tile([128, 4, 256], F32, tag="cps")
                for wpos in range(4):
                    for cc in range(2):
                        for kh in range(3):
                            nc.tensor.matmul(
                                ps[:, wpos, 0:NG],
                                stat(wu, cc, kh, wpos, occ),
                                tws[cc][:, wpos, kh:kh + TR, :],
                                start=(cc == 0 and kh == 0),
                                stop=(cc == 1 and kh == 2))
                sink(occ, t, ps)

    # Y-stage: with u2 slab built negated:
    #   [e1,o1] = m[(0,2)] + bcast(m1);  [e2,o2] = [e1,o1] - m[(2,3)]
    #   e2 = M0+M1+M2 (even outputs), o2 = M1-M2-M3 (odd outputs)
    def ystage(ps, t=0):
        m = stage_p.tile([128, 4, NG], BF16, tag="mev", bufs=3)
        nc.scalar.copy(m[:], ps[:, :, 0:NG])
        m02 = m[:].rearrange("p (j two) n -> p j two n", two=2)[:, :, 0]
        eo1 = stage_p.tile([128, 2, NG], BF16, tag="eo1", bufs=1)
        nc.vector.tensor_add(
            eo1[:], m02, m[:, 1].unsqueeze(1).broadcast_to([128, 2, NG]))
        eo2 = stage_p.tile([128, 2, NG], BF16, tag="eo2")
        nc.vector.tensor_sub(eo2[:], eo1[:], m[:, 2:4])
        return eo2

    def sink1(occ, t, ps):
        r0 = TR * t + 1
        eo2 = ystage(ps, t)
        # even outputs -> o-plane, odd outputs -> e-plane, cols 1..28
        nc.scalar.activation(
            yeo[occ][:, :, r0:r0 + TR, 1:TC + 1],
            eo2[:].rearrange("p a (h w) -> p a h w", h=TR),
            AF.Relu, bias=bnb_sb[0][occ][:], scale=1.0)

    def make_sink2(s, xplanes):
        def sink2(occ, t, ps):
            r0 = TR * t + 1
            eo2 = ystage(ps, t)
            rx = xplanes[occ][:, :, r0:r0 + TR, 1:TC + 1]
            eo3 = stage_p.tile([128, 2, TR, TC], BF16, tag="eo3")
            nc.vector.tensor_add(
                eo3[:], eo2[:].rearrange("p a (h w) -> p a h w", h=TR), rx)
            os_ = stage_p.tile([128, 2, TR, TC], BF16, tag="ostg", bufs=3)
            nc.scalar.activation(os_[:], eo3[:], AF.Relu,
                                 bias=bnb_sb[1][occ][:], scale=1.0)
            nc.sync.dma_start(outd[s, occ][:, :, TR * t:TR * t + TR, :],
                              os_[:])
        return sink2

    # ---- main pipeline ----
    ops0 = []
    xp = load_x_ops(0, ops0)
    # x DMAs first; basis DMAs queue behind them
    ops0[0]()
    ops0[1]()
    for i in range(4):
        load_basis(0, i)
    # paced warmup fills: PE chews on x planes while gap/avec compute
    for _ in range(8):
        fill(xp[0][:, 0, 1:11, 1:29])
        fill(xp[1][:, 0, 1:11, 1:29])
    for f in ops0[2:]:
        f()
    for _ in range(6):
        fill(garb[:, 0:512])
    opsA = []
    avt2_w1 = gen_weights_a_ops(0, 0, opsA)
    for f in opsA:
        f()
    # V strips for conv1(0) come BEFORE the combine in DVE program order
    pre1 = [buildV(xp, 0), buildV(xp, 1)]
    opsB = []
    w1 = gen_weights_b_ops(0, avt2_w1, opsB, pace=True)
    for f in opsB:
        f()
    for i in range(4):
        load_basis(1, i)
    load_deferred_consts()
    opsC = []
    avt2_w2 = gen_weights_a_ops(1, 0, opsC)
    for f in opsC:
        f()

    # Weight-gen windows span 1.5 convs: the occ-0 half of each gen_b runs
    # in the tail groups (5-6) of the conv BEFORE the conv that hides the
    # occ-1 half (groups 0-1), so the dma-transposes never land on the
    # consuming conv's start.
    opsB2 = []
    w2_cur = gen_weights_b_ops(1, avt2_w2, opsB2)
    for f in opsB2[:12]:                    # o0-half right at startup
        f()
    w2_rest = opsB2[12:]

    for s in range(BL):
        w2 = w2_cur
        # conv1(s): o1-half of gen_b(w2,s) [0-1], conv2's V0/V1 [2-3],
        # next x DMA [0], gap [2-3], gen_a(w1,s+1) [3],
        # o0-half of gen_b(w1,s+1) [5-6]
        sched1 = [[] for _ in range(NT)]
        for idx, f in enumerate(w2_rest):
            sched1[idx // 3].append(f)
        pre2_box = [None, None]
        sched1[2].append(lambda b=pre2_box: b.__setitem__(0, buildV(yeo, 0)))
        sched1[3].append(lambda b=pre2_box: b.__setitem__(1, buildV(yeo, 1)))
        opsB1 = []
        if s + 1 < BL:
            opsX = []
            xp_n = load_x_ops(s + 1, opsX)
            sched1[0].append(opsX[0])
            sched1[0].append(opsX[1])
            for f in opsX[2:-1]:
                sched1[2].append(f)         # gap accum (x DMA long done)
            sched1[3].append(opsX[-1])      # gap matmul
            opsA1 = []
            avt2_w1n = gen_weights_a_ops(0, s + 1, opsA1)
            for f in opsA1:
                sched1[3].append(f)
            w1_n = gen_weights_b_ops(0, avt2_w1n, opsB1)
            for idx, f in enumerate(opsB1[:12]):
                sched1[3 + idx // 3].append(f)
        conv(w1, xp, sink1, sched1, pre=pre1)

        # conv2(s): o1-half of gen_b(w1,s+1) [0-1], gen_a(w2,s+1) [1],
        # conv1(s+1)'s V0/V1 [2-3], o0-half of gen_b(w2,s+1) [5-6]
        sched2 = [[] for _ in range(NT)]
        if s + 1 < BL:
            for idx, f in enumerate(opsB1[12:]):
                sched2[idx // 3].append(f)
            opsA2 = []
            avt2_w2n = gen_weights_a_ops(1, s + 1, opsA2)
            for f in opsA2:
                sched2[1].append(f)
            pre1n_box = [None, None]
            sched2[2].append(
                lambda b=pre1n_box: b.__setitem__(0, buildV(xp_n, 0)))
            sched2[3].append(
                lambda b=pre1n_box: b.__setitem__(1, buildV(xp_n, 1)))
            opsB2n = []
            w2_cur = gen_weights_b_ops(1, avt2_w2n, opsB2n)
            for idx, f in enumerate(opsB2n[:12]):
                sched2[3 + idx // 3].append(f)
            w2_rest = opsB2n[12:]
        conv(w2, yeo, make_sink2(s, xp), sched2, pre=pre2_box)

        if s + 1 < BL:
            xp, w1 = xp_n, w1_n
            avt2_w2 = avt2_w2n
            pre1 = pre1n_box

    ctx.close()


_NC_CACHE = {}


def get_program():
    if "nc" not in _NC_CACHE:
        _NC_CACHE["nc"] = build_program()
    return _NC_CACHE["nc"]


def prep_inputs(inputs):
    x = np.asarray(inputs["x"], np.float32)
    f32 = lambda a: np.ascontiguousarray(np.asarray(a, np.float32))
    bf = lambda a: np.ascontiguousarray(
        np.asarray(a, np.float32).astype(ml_dtypes.bfloat16))

    def bn_fold(g, b, m, v):
        sc = np.asarray(g, np.float32) / np.sqrt(np.asarray(v, np.float32) + EPS)
        bia = np.asarray(b, np.float32) - np.asarray(m, np.float32) * sc
        return sc, f32(bia.reshape(2, 128, 1))

    def pack_basis(fc2_w, bn_sc):
        # fc2_w [589824, 4] -> B[i][occ][oc_lo, cc, kw, kh, cin_lo]
        w = np.asarray(fc2_w, np.float32).reshape(256, 256, 3, 3, 4)
        w = w * bn_sc[:, None, None, None, None]       # fold bn scale (per oc)
        w[:, :, :, 1, :] *= 0.5                        # pre-halve kw=1 taps
        # [oc, ic, kh, kw, i] -> [i, oc, kw, kh, ic]
        w = w.transpose(4, 0, 3, 2, 1)
        # oc -> (occ, oc_lo); ic -> (cc, cin_lo)
        w = w.reshape(4, 2, 128, 3, 3, 2, 128).transpose(0, 1, 2, 5, 3, 4, 6)
        return bf(w.reshape(4, 2, 128, 2304))

    s1, b1 = bn_fold(inputs["bn1_g"], inputs["bn1_b"],
                     inputs["bn1_m"], inputs["bn1_v"])
    s2, b2 = bn_fold(inputs["bn2_g"], inputs["bn2_b"],
                     inputs["bn2_m"], inputs["bn2_v"])

    NPIX = H * W
    base = {
        "rwT": f32((np.asarray(inputs["reduce_w"], np.float32).T / NPIX)
                   .reshape(2, 128, 16)),
        "rb": f32(np.asarray(inputs["reduce_b"]).reshape(16, 1)),
        "fc1wT1": bf(np.asarray(inputs["w1_fc1_w"]).T),
        "fc1wT2": bf(np.asarray(inputs["w2_fc1_w"]).T),
        "fc1b1": f32(np.asarray(inputs["w1_fc1_b"]).reshape(32, 128).T),
        "fc1b2": f32(np.asarray(inputs["w2_fc1_b"]).reshape(32, 128).T),
        "bas1": pack_basis(inputs["w1_fc2_w"], s1),
        "bas2": pack_basis(inputs["w2_fc2_w"], s2),
        "bnb1": b1,
        "bnb2": b2,
    }

    # host-prepadded planes: o-plane[j] = xpad[2j-1] (x even cols, at 1..28),
    # e-plane[j] = xpad[2j] (x odd cols at 1..28; col 0 = xpad[0] = 0)
    xb = x.astype(ml_dtypes.bfloat16)
    xeo = np.zeros((B, C, 2, RP, PW), ml_dtypes.bfloat16)
    xeo[:, :, 0, 1:RP - 1, 1:TC + 1] = xb[:, :, :, 0::2]
    xeo[:, :, 1, 1:RP - 1, 1:TC + 1] = xb[:, :, :, 1::2]

    in_maps = []
    for i in range(NCORES):
        m = dict(base)
        m["xeo4"] = np.ascontiguousarray(
            xeo[i * BL:(i + 1) * BL].reshape(BL, 2, 128, 2, RP, PW))
        in_maps.append(m)
    return in_maps


def unpack_outputs(results):
    outs = []
    for r in results:
        od = np.asarray(r["outd"], ml_dtypes.bfloat16).astype(np.float32)
        out = np.zeros((BL, 2, 128, H, W), np.float32)
        out[..., 0::2] = od[:, :, :, 0]
        out[..., 1::2] = od[:, :, :, 1]
        outs.append(out.reshape(BL, C, H, W))
    return np.concatenate(outs, axis=0)


def kernel(**inputs):
    in_maps = prep_inputs(inputs)
    nc = get_program()
    res = bass_utils.run_bass_kernel_spmd(nc, in_maps,
                                          core_ids=list(range(NCORES)))
    return unpack_outputs(res.results)


# revision 35
# speedup vs baseline: 1.0744x; 1.0744x over previous
"""Trainium2 Bass kernel for nn_BasicBlock (WeightNet/CondConv-style block).

Data parallel over batch: 32 samples -> 8 cores x 4 samples.

Conv: 1D Winograd F(2,3) along W (M-form). Even/odd column planes
(host-prepadded) -> 4 V planes per row-strip via DVE/Pool adds.

Weight generation runs in W^T orientation ([oc partitions, (cc,kw,kh,cin)
free]) where the rank-4 mixing coefficients are *per-partition* f32
scalars: 4x tensor_scalar_mul (4x DVE mode) + 3 adds replaces the old
broadcast-tile + 7 tensor_tensor passes, and the aexp broadcast matmuls
disappear.  The 1D Winograd U-transform runs as slab ops (uu/u1/u2) in
W^T, then four dma_start_transpose ops per (occ,cc) drop the stationary
tiles [cin, 12, oc] directly (kw0 | u1 | u2 | kw2 slabs).  BN scale and
the halving of the centre tap are folded into the host-packed basis.

The Y-stage runs as: one ACT evacuation of the 4 M psum planes, two
merged 2-lane DVE ops, one merged relu(+bias) ACT into the next conv's
input planes (U2 slab is built negated so eo2 = eo1 - m[2:4] works).
"""

import sys

sys.path.insert(0, "/opt/trn_rl_repo")

import numpy as np
import ml_dtypes

import concourse.bass as bass
import concourse.tile as tile
from concourse import bacc, mybir
from concourse import bass_utils

F32 = mybir.dt.float32
BF16 = mybir.dt.bfloat16
AF = mybir.ActivationFunctionType
ALU = mybir.AluOpType

B, C, H, W = 32, 256, 56, 56
NCORES = 8
BL = B // NCORES          # samples per core
RP = H + 2                # padded rows: 58
TC = W // 2               # tile cols: 28
PW = TC + 2               # plane width: 30
NT, TR = 7, 8             # row-groups
NG = TR * TC              # 224 cols per M plane slice
EPS = 1e-5


def build_program():
    nc = bacc.Bacc("TRN2", target_bir_lowering=False, debug=False,
                   num_devices=NCORES)

    # host-prepadded planes: [s, cc, 128, (o-plane, e-plane), RP, PW]
    xeo4 = nc.dram_tensor("xeo4", [BL, 2, 128, 2, RP, PW], BF16,
                          kind="ExternalInput").ap()
    # out: [s, occ, 128, (even-cols, odd-cols), H, TC]
    outd = nc.dram_tensor("outd", [BL, 2, 128, 2, H, TC], BF16,
                          kind="ExternalOutput").ap()
    rwT = nc.dram_tensor("rwT", [2, 128, 16], F32, kind="ExternalInput").ap()
    rb = nc.dram_tensor("rb", [16, 1], F32, kind="ExternalInput").ap()
    fc1wT = [nc.dram_tensor(f"fc1wT{n}", [16, 4096], BF16,
                            kind="ExternalInput").ap() for n in (1, 2)]
    fc1b = [nc.dram_tensor(f"fc1b{n}", [128, 32], F32,
                           kind="ExternalInput").ap() for n in (1, 2)]
    # W^T basis: [i, occ, 128 oc, (cc 2, kw 3, kh 3, cin 128)]
    basd = [nc.dram_tensor(f"bas{n}", [4, 2, 128, 2304], BF16,
                           kind="ExternalInput").ap() for n in (1, 2)]
    bnb = [nc.dram_tensor(f"bnb{n}", [2, 128, 1], F32,
                          kind="ExternalInput").ap() for n in (1, 2)]

    with tile.TileContext(nc) as tc:
        build_body(tc, xeo4, outd, rwT, rb, fc1wT, fc1b, basd, bnb)

    nc.compile()
    return nc


def build_body(tc, xeo4, outd, rwT, rb, fc1wT, fc1b, basd, bnb):
    nc = tc.nc
    from contextlib import ExitStack
    ctx = ExitStack()

    cpool = ctx.enter_context(tc.tile_pool(name="consts", bufs=1))
    xeo_p = ctx.enter_context(tc.tile_pool(name="xeo", bufs=2))
    twS_p = ctx.enter_context(tc.tile_pool(name="twS", bufs=6))
    wt9_p = ctx.enter_context(tc.tile_pool(name="wt9", bufs=1))
    wst_p = ctx.enter_context(tc.tile_pool(name="wst", bufs=2))
    usl_p = ctx.enter_context(tc.tile_pool(name="uslab", bufs=2))
    ctmp_p = ctx.enter_context(tc.tile_pool(name="ctmp", bufs=2))
    small_p = ctx.enter_context(tc.tile_pool(name="small", bufs=2))
    stage_p = ctx.enter_context(tc.tile_pool(name="stage", bufs=2))
    psum_p = ctx.enter_context(tc.tile_pool(name="psum", bufs=3, space="PSUM"))
    psmall_p = ctx.enter_context(tc.tile_pool(name="psmall", bufs=1,
                                              space="PSUM"))
    dram_p = ctx.enter_context(tc.tile_pool(name="dscratch", bufs=2,
                                            space="DRAM"))

    # ---- resident constants ----
    # basis tiles: [wn][i][occ] -> [128 oc, 2 cc, 9 (kw,kh), 128 cin]
    bas_sb = []
    for n in range(2):
        per = []
        for i in range(4):
            per.append([cpool.tile([128, 2, 9, 128], BF16,
                                   tag=f"bas{n}{i}{o}", name=f"bas{n}{i}{o}")
                        for o in range(2)])
        bas_sb.append(per)

    def load_basis(n, i):
        for o in range(2):
            nc.sync.dma_start(
                bas_sb[n][i][o][:],
                basd[n][i, o].rearrange("p (c k m) -> p c k m", c=2, k=9))

    rwT_sb = []
    for c in range(2):
        t = cpool.tile([128, 16], F32, tag=f"rwT{c}")
        nc.sync.dma_start(t[:], rwT[c])
        rwT_sb.append(t)
    rb_sb = cpool.tile([16, 1], F32, tag="rb")
    nc.sync.dma_start(rb_sb[:], rb)
    fc1b_sb, bnb_sb = [], []
    fc1w_t = cpool.tile([16, 4096], BF16, tag="fc1w")
    for n in range(2):
        t = cpool.tile([128, 32], F32, tag=f"fc1b{n}")
        if n == 0:
            nc.sync.dma_start(t[:], fc1b[n])
        fc1b_sb.append(t)
        tb = [cpool.tile([128, 1], F32, tag=f"bnb{n}{c}", name=f"bnbt{n}{c}")
              for c in range(2)]
        bnb_sb.append(tb)

    def load_deferred_consts():
        nc.sync.dma_start(fc1b_sb[1][:], fc1b[1])
        for n in range(2):
            for c in range(2):
                nc.sync.dma_start(bnb_sb[n][c][:], bnb[n][c])

    gap16 = cpool.tile([16, BL], BF16, tag="gap16")
    garb = cpool.tile([128, RP * PW], BF16, tag="garb")

    # PE filler: tiny matmuls that keep the tensor engine "hot" (p-state)
    # during unavoidable PE gaps.  fill(dep) emits a matmul whose moving
    # operand is a slice of `dep`, pacing it behind that tile's producer.
    fill_a = cpool.tile([128, 2], BF16, tag="fill_a")
    nc.gpsimd.memset(fill_a[:], 0.0)

    # conv1 output planes (= conv2 input planes): [128, (o,e), RP, PW]
    # zeroed once; sinks only ever write rows 1..56, cols 1..28
    yeo = [cpool.tile([128, 2, RP, PW], BF16, tag=f"yeo{c}", name=f"yeo{c}")
           for c in range(2)]
    for c in range(2):
        nc.gpsimd.memset(yeo[c][:], 0.0)
    fill_ps = psmall_p.tile([128, 512], F32, tag="fill_ps", name="fill_ps")

    def fill(dep_ap, n=1):
        sz = 1
        for d in dep_ap.shape[1:]:
            sz *= d
        for _ in range(n):
            nc.tensor.matmul(fill_ps[0:2, 0:sz], fill_a[:],
                             dep_ap, start=True, stop=True)

    # ---- weight generation ----
    def gen_weights_a_ops(wn, s, ops):
        """sigmoid(fc1(gap)) -> DRAM roundtrip -> per-partition scalar
        tile avt2 [128 oc_lo, occ, (q,i)]."""
        apsg = psmall_p.tile([128, 33], F32, tag="avec_ps")
        aps = apsg[:, 0:32]
        avt = small_p.tile([128, 32], F32, tag="avtmp")
        avecf = small_p.tile([128, 32], F32, tag="avecf")
        avd = dram_p.tile([4096], F32, tag="avd")
        avt2 = small_p.tile([128, 2, 16], F32, tag="avt2")

        ops.append(lambda: nc.sync.dma_start(fc1w_t[:], fc1wT[wn]))

        def avec_mms(j0):
            def f():
                for j in range(j0, j0 + 8):
                    nc.tensor.matmul(aps[:, j:j + 1],
                                     fc1w_t[:, 128 * j:128 * (j + 1)],
                                     gap16[:, s:s + 1],
                                     start=True, stop=True)
            return f
        for j0 in range(0, 32, 8):
            ops.append(avec_mms(j0))
        ops.append(lambda: nc.vector.tensor_add(avt[:], aps, fc1b_sb[wn][:]))
        ops.append(lambda: nc.scalar.activation(avecf[:], avt[:], AF.Sigmoid))
        # roundtrip: a[l], l = oc*16 + q*4 + i  ->  avt2[p, occ, (q i)]
        ops.append(lambda: nc.sync.dma_start(
            avd[:].rearrange("(j p) -> p j", p=128), avecf[:]))
        ops.append(lambda: nc.sync.dma_start(
            avt2[:], avd[:].rearrange("(o p c) -> p o c", o=2, c=16)))
        return avt2

    def gen_weights_b_ops(wn, avt2, ops, pace=False):
        """W^T combine -> U slabs -> dma-transposes -> stationary tiles.

        Returns wst[cc][occ] tiles [128 cin, 12 (wpos,kh), 128 oc]."""
        wt9 = [wt9_p.tile([128, 2, 9, 128], BF16, tag=f"wt9{o}",
                          name=f"wt9_{o}") for o in range(2)]
        wst = [[wst_p.tile([128, 12, 128], BF16, tag=f"wst{c}{o}",
                           name=f"wst{c}{o}") for o in range(2)]
               for c in range(2)]

        # combine: wt9[occ][:, cc, :, q*64:...] = sum_i bas_i * a_i
        def combine(o, c, qh):
            q = 2 * c + qh
            sl = (slice(None), c, slice(None), slice(64 * qh, 64 * qh + 64))

            def sca(i):
                return avt2[:, o, 4 * q + i:4 * q + i + 1]

            tm = [ctmp_p.tile([128, 9, 64], BF16, tag=f"ctm{k % 2}",
                              name=f"ctm{k}") for k in range(3)]

            def f1():
                acc = wt9[o][sl]
                nc.vector.tensor_scalar_mul(acc, bas_sb[wn][0][o][sl], sca(0))
                for i in range(1, 4):
                    nc.vector.tensor_scalar_mul(tm[i - 1][:],
                                                bas_sb[wn][i][o][sl], sca(i))

            def f2():
                acc = wt9[o][sl]
                nc.vector.tensor_add(acc, acc, tm[0][:])
                nc.vector.tensor_add(acc, acc, tm[1][:])
                nc.vector.tensor_add(acc, acc, tm[2][:])
                if pace:
                    fill(wt9[o][:, c, 0:4, 64 * qh:64 * qh + 64], 2)
            return f1, f2
        # U slabs + transposes:  kw0 | u1 | u2(neg) | kw2
        #   uuh = 0.5*(W0+W2);  u1 = uuh + W1h ;  u2 = W1h - uuh
        # (W1h pre-halved on host.)
        def mk_uops_tps(o, c):
            uu = usl_p.tile([128, 3, 128], BF16, tag="uu", bufs=1,
                            name=f"uu{c}{o}")
            uuh = usl_p.tile([128, 3, 128], BF16, tag="uuh", bufs=1,
                             name=f"uuh{c}{o}")
            u1s = usl_p.tile([128, 3, 128], BF16, tag="u1s",
                             name=f"u1s{c}{o}")
            u2s = usl_p.tile([128, 3, 128], BF16, tag="u2s",
                             name=f"u2s{c}{o}")
            kw0 = wt9[o][:, c, 0:3, :]
            kw1 = wt9[o][:, c, 3:6, :]
            kw2 = wt9[o][:, c, 6:9, :]

            def uops():
                nc.gpsimd.tensor_add(uu[:], kw0, kw2)
                nc.vector.tensor_scalar_mul(uuh[:], uu[:], 0.5)
                nc.gpsimd.tensor_add(u1s[:], uuh[:], kw1)
                nc.gpsimd.tensor_sub(u2s[:], kw1, uuh[:])
                if pace:
                    fill(u1s[:, :, 0:128], 2)
                    fill(u2s[:, :, 0:128], 2)

            def tps():
                w = wst[c][o]
                nc.sync.dma_start_transpose(w[:, 0:3, :], kw0)
                nc.sync.dma_start_transpose(w[:, 3:6, :], u1s[:])
                nc.sync.dma_start_transpose(w[:, 6:9, :], u2s[:])
                nc.sync.dma_start_transpose(w[:, 9:12, :], kw2)
            return uops, tps

        for o in range(2):
            for c in range(2):
                f1a, f2a = combine(o, c, 0)
                f1b, f2b = combine(o, c, 1)
                uops, tps = mk_uops_tps(o, c)
                ops.extend([f1a, f2a, f1b, f2b, uops, tps])
        return wst

    def stat(wu, cc, kh, wpos, occ):
        return wu[cc][occ][:, 3 * wpos + kh, :]

    # ---- x loading + gap ----
    def load_x_ops(s, ops):
        planes = []
        for c in range(2):
            te = xeo_p.tile([128, 2, RP, PW], BF16, tag=f"xeo{c}",
                            name=f"xeo{c}")
            planes.append(te)
        gacc = []
        for c in range(2):
            ga = [small_p.tile([128, 1], F32, tag=f"gacc{c}{a}",
                               name=f"gacc{c}{a}") for a in range(2)]
            gs = small_p.tile([128, 1], F32, tag=f"gsum{c}", name=f"gsum{c}")
            gacc.append((ga, gs))
        apsg = psmall_p.tile([128, 33], F32, tag="avec_ps", name="gapps")
        gps = apsg[0:16, 32:33]

        for c in range(2):
            ops.append(lambda c=c: nc.sync.dma_start(planes[c][:], xeo4[s, c]))
        for c in range(2):
            for a in range(2):
                ops.append(lambda c=c, a=a: nc.scalar.activation(
                    garb[:], planes[c][:, a].rearrange("p h w -> p (h w)"),
                    AF.Copy, accum_out=gacc[c][0][a][:]))
            ops.append(lambda c=c: nc.vector.tensor_add(
                gacc[c][1][:], gacc[c][0][0][:], gacc[c][0][1][:]))

        def gapmm():
            for c in range(2):
                nc.tensor.matmul(gps, rwT_sb[c][:], gacc[c][1][:],
                                 start=(c == 0), stop=(c == 1))
            nc.scalar.activation(gap16[:, s:s + 1], gps, AF.Identity,
                                 bias=rb_sb[:], scale=1.0)
        ops.append(gapmm)
        return planes

    # ---- conv: per-t-group V strips + matmuls + sink ----
    # V planes (from o-plane ol/orr, e-plane el/er):
    #   V0 = el - er; V1 = ol + er; V2 = er - ol; V3 = ol - orr
    def buildV(planes, t):
        r0 = TR * t
        tws = []
        for c in range(2):
            st = twS_p.tile([128, 4, TR + 2, TC], BF16, tag=f"tws{c}",
                            name=f"tws{c}")
            e = planes[c][:, 1, r0:r0 + TR + 2, :]
            o = planes[c][:, 0, r0:r0 + TR + 2, :]
            el = e[:, :, 0:TC]
            er = e[:, :, 1:TC + 1]
            ol = o[:, :, 1:TC + 1]
            orr = o[:, :, 2:TC + 2]
            nc.vector.tensor_sub(st[:, 0], el, er)
            nc.vector.tensor_add(st[:, 1], ol, er)
            nc.gpsimd.tensor_sub(st[:, 2], er, ol)
            nc.gpsimd.tensor_sub(st[:, 3], ol, orr)
            tws.append(st)
        return tws

    def conv(wu, planes, sink, sched=None, pre=None, stagger=False):
        """V strips are built two groups ahead (before the current group's
        ystage ops, so DVE/Pool chew them while PE runs group t).
        sched[t] is a list of background thunks emitted at group t.
        pre = [tws(0), tws(1)] built by the previous conv's stream.
        stagger: emit (t0,o0),(t1,o0),(t0,o1),(t1,o1) so the occ-0
        matmuls run while the occ-1 stationaries are still being built
        (startup only)."""
        if pre is None:
            pre = [buildV(planes, 0), buildV(planes, 1)]
        pipe = list(pre)
        tws_hist = {}

        def mmgrp(tws, t, occ):
            ps = psum_p.tile([128, 4, 256], F32, tag="cps")
            for wpos in range(4):
                for cc in range(2):
                    for kh in range(3):
                        nc.tensor.matmul(
                            ps[:, wpos, 0:NG],
                            stat(wu, cc, kh, wpos, occ),
                            tws[cc][:, wpos, kh:kh + TR, :],
                            start=(cc == 0 and kh == 0),
                            stop=(cc == 1 and kh == 2))
            sink(occ, t, ps)

        for t in range(NT):
            if t + 2 < NT:
                pipe.append(buildV(planes, t + 2))
            if sched:
                for f in sched[t]:
                    f()
            tws = pipe.pop(0)
            tws_hist[t] = tws
            if stagger and t == 0:
                mmgrp(tws, 0, 0)
            elif stagger and t == 1:
                mmgrp(tws, 1, 0)
                mmgrp(tws_hist[0], 0, 1)
                mmgrp(tws, 1, 1)
            else:
                mmgrp(tws, t, 0)
                mmgrp(tws, t, 1)

    # Y-stage: with u2 slab built negated:
    #   [e1,o1] = m[(0,2)] + bcast(m1);  [e2,o2] = [e1,o1] - m[(2,3)]
    #   e2 = M0+M1+M2 (even outputs), o2 = M1-M2-M3 (odd outputs)
    def ystage(ps, t=0):
        m = stage_p.tile([128, 4, NG], BF16, tag="mev", bufs=3)
        nc.scalar.copy(m[:], ps[:, :, 0:NG])
        m02 = m[:].rearrange("p (j two) n -> p j two n", two=2)[:, :, 0]
        eo1 = stage_p.tile([128, 2, NG], BF16, tag="eo1", bufs=1)
        nc.vector.tensor_add(
            eo1[:], m02, m[:, 1].unsqueeze(1).broadcast_to([128, 2, NG]))
        eo2 = stage_p.tile([128, 2, NG], BF16, tag="eo2")
        nc.vector.tensor_sub(eo2[:], eo1[:], m[:, 2:4])
        return eo2

    def sink1(occ, t, ps):
        r0 = TR * t + 1
        eo2 = ystage(ps, t)
        # even outputs -> o-plane, odd outputs -> e-plane, cols 1..28
        nc.scalar.activation(
            yeo[occ][:, :, r0:r0 + TR, 1:TC + 1],
            eo2[:].rearrange("p a (h w) -> p a h w", h=TR),
            AF.Relu, bias=bnb_sb[0][occ][:], scale=1.0)

    def make_sink2(s, xplanes):
        def sink2(occ, t, ps):
            r0 = TR * t + 1
            eo2 = ystage(ps, t)
            rx = xplanes[occ][:, :, r0:r0 + TR, 1:TC + 1]
            eo3 = stage_p.tile([128, 2, TR, TC], BF16, tag="eo3")
            nc.vector.tensor_add(
                eo3[:], eo2[:].rearrange("p a (h w) -> p a h w", h=TR), rx)
            os_ = stage_p.tile([128, 2, TR, TC], BF16, tag="ostg", bufs=3)
            nc.scalar.activation(os_[:], eo3[:], AF.Relu,
                                 bias=bnb_sb[1][occ][:], scale=1.0)
            nc.sync.dma_start(outd[s, occ][:, :, TR * t:TR * t + TR, :],
                              os_[:])
        return sink2

    # ---- main pipeline ----
    ops0 = []
    xp = load_x_ops(0, ops0)
    # x DMAs first; basis DMAs queue behind them
    ops0[0]()
    ops0[1]()
    for i in range(4):
        load_basis(0, i)
    # paced warmup fills: PE chews on x planes while gap/avec compute
    for _ in range(8):
        fill(xp[0][:, 0, 1:11, 1:29])
        fill(xp[1][:, 0, 1:11, 1:29])
    for f in ops0[2:]:
        f()
    for _ in range(6):
        fill(garb[:, 0:512])
    opsA = []
    avt2_w1 = gen_weights_a_ops(0, 0, opsA)
    for f in opsA:
        f()
    # V strips for conv1(0) come BEFORE the combine in DVE program order
    pre1 = [buildV(xp, 0), buildV(xp, 1)]
    opsB = []
    w1 = gen_weights_b_ops(0, avt2_w1, opsB, pace=True)
    for f in opsB:
        f()
    for i in range(4):
        load_basis(1, i)
    load_deferred_consts()
    opsC = []
    avt2_w2 = gen_weights_a_ops(1, 0, opsC)
    for f in opsC:
        f()

    # Weight-gen windows: the occ-0 half of each gen_b is emitted at the
    # boundary right before its hiding conv (K_O0=bnd) or in the tail
    # groups of the previous conv (K_O0=tail); the occ-1 half goes into
    # the hiding conv's sched groups.
    import os as _os
    K_O0 = _os.environ.get('K_O0', 'bnd')
    K_O1G0 = int(_os.environ.get('K_O1G0', '1'))
    K_O1RATE = int(_os.environ.get('K_O1RATE', '3'))

    def place_o1(thunks, sched):
        for idx, f in enumerate(thunks):
            sched[min(K_O1G0 + idx // K_O1RATE, NT - 1)].append(f)

    opsB2 = []
    w2_cur = gen_weights_b_ops(1, avt2_w2, opsB2)
    # s=0: no previous conv to host the o0-half; spread everything
    w2_first, w2_rest = [], opsB2

    for s in range(BL):
        w2 = w2_cur
        sched1 = [[] for _ in range(NT)]
        if s == 0:
            for idx, f in enumerate(w2_rest):
                sched1[min(1 + idx // 4, NT - 1)].append(f)
        else:
            place_o1(w2_rest, sched1)
        pre2_box = [None, None]
        sched1[2].append(lambda b=pre2_box: b.__setitem__(0, buildV(yeo, 0)))
        sched1[3].append(lambda b=pre2_box: b.__setitem__(1, buildV(yeo, 1)))
        opsB1 = []
        if s + 1 < BL:
            opsX = []
            xp_n = load_x_ops(s + 1, opsX)
            sched1[0].append(opsX[0])
            sched1[0].append(opsX[1])
            for f in opsX[2:-1]:
                sched1[2].append(f)         # gap accum (x DMA long done)
            sched1[3].append(opsX[-1])      # gap matmul
            opsA1 = []
            avt2_w1n = gen_weights_a_ops(0, s + 1, opsA1)
            for f in opsA1:
                sched1[3].append(f)
            w1_n = gen_weights_b_ops(0, avt2_w1n, opsB1)
            if K_O0 == 'tail':
                for idx, f in enumerate(opsB1[:12]):
                    sched1[5 + idx // 6].append(f)
        for f in w2_first:                  # boundary slot (K_O0=bnd)
            f()
        conv(w1, xp, sink1, sched1, pre=pre1, stagger=False)

        sched2 = [[] for _ in range(NT)]
        if s + 1 < BL:
            place_o1(opsB1[12:], sched2)
            opsA2 = []
            avt2_w2n = gen_weights_a_ops(1, s + 1, opsA2)
            for f in opsA2:
                sched2[1].append(f)
            pre1n_box = [None, None]
            sched2[2].append(
                lambda b=pre1n_box: b.__setitem__(0, buildV(xp_n, 0)))
            sched2[3].append(
                lambda b=pre1n_box: b.__setitem__(1, buildV(xp_n, 1)))
            opsB2n = []
            w2_cur = gen_weights_b_ops(1, avt2_w2n, opsB2n)
            if K_O0 == 'tail':
                for idx, f in enumerate(opsB2n[:12]):
                    sched2[5 + idx // 6].append(f)
                w2_first = []
            else:
                w2_first = opsB2n[:12]
            w2_rest = opsB2n[12:]
            if K_O0 == 'bnd':
                for f in opsB1[:12]:        # boundary before conv2
                    f()
        conv(w2, yeo, make_sink2(s, xp), sched2, pre=pre2_box)

        if s + 1 < BL:
            xp, w1 = xp_n, w1_n
            avt2_w2 = avt2_w2n
            pre1 = pre1n_box

    ctx.close()


_NC_CACHE = {}


def get_program():
    if "nc" not in _NC_CACHE:
        _NC_CACHE["nc"] = build_program()
    return _NC_CACHE["nc"]


def prep_inputs(inputs):
    x = np.asarray(inputs["x"], np.float32)
    f32 = lambda a: np.ascontiguousarray(np.asarray(a, np.float32))
    bf = lambda a: np.ascontiguousarray(
        np.asarray(a, np.float32).astype(ml_dtypes.bfloat16))

    def bn_fold(g, b, m, v):
        sc = np.asarray(g, np.float32) / np.sqrt(np.asarray(v, np.float32) + EPS)
        bia = np.asarray(b, np.float32) - np.asarray(m, np.float32) * sc
        return sc, f32(bia.reshape(2, 128, 1))

    def pack_basis(fc2_w, bn_sc):
        # fc2_w [589824, 4] -> B[i][occ][oc_lo, cc, kw, kh, cin_lo]
        w = np.asarray(fc2_w, np.float32).reshape(256, 256, 3, 3, 4)
        w = w * bn_sc[:, None, None, None, None]       # fold bn scale (per oc)
        w[:, :, :, 1, :] *= 0.5                        # pre-halve kw=1 taps
        # [oc, ic, kh, kw, i] -> [i, oc, kw, kh, ic]
        w = w.transpose(4, 0, 3, 2, 1)
        # oc -> (occ, oc_lo); ic -> (cc, cin_lo)
        w = w.reshape(4, 2, 128, 3, 3, 2, 128).transpose(0, 1, 2, 5, 3, 4, 6)
        return bf(w.reshape(4, 2, 128, 2304))

    s1, b1 = bn_fold(inputs["bn1_g"], inputs["bn1_b"],
                     inputs["bn1_m"], inputs["bn1_v"])
    s2, b2 = bn_fold(inputs["bn2_g"], inputs["bn2_b"],
                     inputs["bn2_m"], inputs["bn2_v"])

    NPIX = H * W
    base = {
        "rwT": f32((np.asarray(inputs["reduce_w"], np.float32).T / NPIX)
                   .reshape(2, 128, 16)),
        "rb": f32(np.asarray(inputs["reduce_b"]).reshape(16, 1)),
        "fc1wT1": bf(np.asarray(inputs["w1_fc1_w"]).T),
        "fc1wT2": bf(np.asarray(inputs["w2_fc1_w"]).T),
        "fc1b1": f32(np.asarray(inputs["w1_fc1_b"]).reshape(32, 128).T),
        "fc1b2": f32(np.asarray(inputs["w2_fc1_b"]).reshape(32, 128).T),
        "bas1": pack_basis(inputs["w1_fc2_w"], s1),
        "bas2": pack_basis(inputs["w2_fc2_w"], s2),
        "bnb1": b1,
        "bnb2": b2,
    }

    # host-prepadded planes: o-plane[j] = xpad[2j-1] (x even cols, at 1..28),
    # e-plane[j] = xpad[2j] (x odd cols at 1..28; col 0 = xpad[0] = 0)
    xb = x.astype(ml_dtypes.bfloat16)
    xeo = np.zeros((B, C, 2, RP, PW), ml_dtypes.bfloat16)
    xeo[:, :, 0, 1:RP - 1, 1:TC + 1] = xb[:, :, :, 0::2]
    xeo[:, :, 1, 1:RP - 1, 1:TC + 1] = xb[:, :, :, 1::2]

    in_maps = []
    for i in range(NCORES):
        m = dict(base)
        m["xeo4"] = np.ascontiguousarray(
            xeo[i * BL:(i + 1) * BL].reshape(BL, 2, 128, 2, RP, PW))
        in_maps.append(m)
    return in_maps


def unpack_outputs(results):
    outs = []
    for r in results:
        od = np.asarray(r["outd"], ml_dtypes.bfloat16).astype(np.float32)
        out = np.zeros((BL, 2, 128, H, W), np.float32)
        out[..., 0::2] = od[:, :, :, 0]
        out[..., 1::2] = od[:, :, :, 1]
        outs.append(out.reshape(BL, C, H, W))
    return np.concatenate(outs, axis=0)


def kernel(**inputs):
    in_maps = prep_inputs(inputs)
    nc = get_program()
    res = bass_utils.run_bass_kernel_spmd(nc, in_maps,
                                          core_ids=list(range(NCORES)))
    return unpack_outputs(res.results)


# revision 59
# speedup vs baseline: 1.0834x; 1.0084x over previous
"""Trainium2 Bass kernel for nn_BasicBlock (WeightNet/CondConv-style block).

Data parallel over batch: 32 samples -> 8 cores x 4 samples.

Conv strategy: 1D Winograd F(2,3) along W (M-form), 1.5x fewer PE
cycles than direct conv. Even/odd column planes (host-prepadded, packed
[o-plane, e-plane] per chunk) -> 4 V planes per row-strip via DVE adds.
Generated weights (k-space combine as baseline, split DVE/Pool) get a
cheap 1D U-transform; U2 is built NEGATED so the whole Y-stage runs as:
one ACT evacuation of all 4 M psum planes, two merged 2-lane DVE ops,
one merged relu(+bias) ACT into the next conv's input planes. BN scale
and the 0.5 of the F(2,3) G-matrix are folded into the host basis.
"""

import sys

sys.path.insert(0, "/opt/trn_rl_repo")

import numpy as np
import ml_dtypes

import concourse.bass as bass
import concourse.tile as tile
from concourse import bacc, mybir
from concourse import bass_utils

F32 = mybir.dt.float32
BF16 = mybir.dt.bfloat16
AF = mybir.ActivationFunctionType
ALU = mybir.AluOpType

B, C, H, W = 32, 256, 56, 56
NCORES = 8
BL = B // NCORES          # samples per core
RP = H + 2                # padded rows: 58
TC = W // 2               # tile cols: 28
PW = TC + 2               # plane width: 30
NT, TR = 7, 8             # row-groups
NG = TR * TC              # 224 cols per M plane slice
EPS = 1e-5


def build_program():
    nc = bacc.Bacc("TRN2", target_bir_lowering=False, debug=False,
                   num_devices=NCORES)

    # host-prepadded planes: [s, cc, 128, (o-plane, e-plane), RP, PW]
    xeo4 = nc.dram_tensor("xeo4", [BL, 2, 128, 2, RP, PW], BF16,
                          kind="ExternalInput").ap()
    # out: [s, occ, 128, (even-cols, odd-cols), H, TC]
    outd = nc.dram_tensor("outd", [BL, 2, 128, 2, H, TC], BF16,
                          kind="ExternalOutput").ap()
    rwT = nc.dram_tensor("rwT", [2, 128, 16], F32, kind="ExternalInput").ap()
    rb = nc.dram_tensor("rb", [16, 1], F32, kind="ExternalInput").ap()
    fc1wT = [nc.dram_tensor(f"fc1wT{n}", [16, 4096], BF16,
                            kind="ExternalInput").ap() for n in (1, 2)]
    fc1b = [nc.dram_tensor(f"fc1b{n}", [128, 32], F32,
                           kind="ExternalInput").ap() for n in (1, 2)]
    w2p = [nc.dram_tensor(f"w2p{n}", [2, 128, 4 * 9 * 256], BF16,
                          kind="ExternalInput").ap() for n in (1, 2)]
    bnb = [nc.dram_tensor(f"bnb{n}", [2, 128, 1], F32,
                          kind="ExternalInput").ap() for n in (1, 2)]

    with tile.TileContext(nc) as tc:
        build_body(tc, xeo4, outd, rwT, rb, fc1wT, fc1b, w2p, bnb)

    nc.compile()
    return nc


def build_body(tc, xeo4, outd, rwT, rb, fc1wT, fc1b, w2p, bnb):
    nc = tc.nc
    from contextlib import ExitStack
    ctx = ExitStack()

    cpool = ctx.enter_context(tc.tile_pool(name="consts", bufs=1))
    xeo_p = ctx.enter_context(tc.tile_pool(name="xeo", bufs=2))
    twS_p = ctx.enter_context(tc.tile_pool(name="twS", bufs=4))
    wg_p = ctx.enter_context(tc.tile_pool(name="wgen", bufs=1))
    wtmp_p = ctx.enter_context(tc.tile_pool(name="wtmp", bufs=2))
    small_p = ctx.enter_context(tc.tile_pool(name="small", bufs=2))
    stage_p = ctx.enter_context(tc.tile_pool(name="stage", bufs=2))
    avlin_p = ctx.enter_context(tc.tile_pool(name="avlinp", bufs=1))
    aexp_p = ctx.enter_context(tc.tile_pool(name="aexp", bufs=1))
    psum_p = ctx.enter_context(tc.tile_pool(name="psum", bufs=3, space="PSUM"))
    psmall_p = ctx.enter_context(tc.tile_pool(name="psmall", bufs=1,
                                              space="PSUM"))
    dram_p = ctx.enter_context(tc.tile_pool(name="dscratch", bufs=2,
                                            space="DRAM"))

    # ---- resident constants ----
    w2sb = []   # [wn][cc][i] -> [128, 2304] bf16 (k-major: 9k x 256oc)
    for n in range(2):
        per = []
        for c in range(2):
            per.append([cpool.tile([128, 2304], BF16, tag=f"w2sb{n}{c}{i}",
                                   name=f"w2sb{n}{c}{i}")
                        for i in range(4)])
        w2sb.append(per)

    def load_w2sb(n):
        for c in range(2):
            for i in range(4):
                nc.sync.dma_start(w2sb[n][c][i][:],
                                  w2p[n][c][:, 2304 * i:2304 * (i + 1)])

    rwT_sb = []
    for c in range(2):
        t = cpool.tile([128, 16], F32, tag=f"rwT{c}")
        nc.sync.dma_start(t[:], rwT[c])
        rwT_sb.append(t)
    rb_sb = cpool.tile([16, 1], F32, tag="rb")
    nc.sync.dma_start(rb_sb[:], rb)
    fc1b_sb, bnb_sb = [], []
    fc1w_t = cpool.tile([16, 4096], BF16, tag="fc1w")
    for n in range(2):
        t = cpool.tile([128, 32], F32, tag=f"fc1b{n}")
        if n == 0:
            nc.sync.dma_start(t[:], fc1b[n])
        fc1b_sb.append(t)
        tb = [cpool.tile([128, 1], F32, tag=f"bnb{n}{c}", name=f"bnbt{n}{c}")
              for c in range(2)]
        bnb_sb.append(tb)

    def load_deferred_consts():
        nc.sync.dma_start(fc1b_sb[1][:], fc1b[1])
        for n in range(2):
            for c in range(2):
                nc.sync.dma_start(bnb_sb[n][c][:], bnb[n][c])

    gap16 = cpool.tile([16, BL], BF16, tag="gap16")
    garb = cpool.tile([128, RP * PW], BF16, tag="garb")
    ones_sb = cpool.tile([1, 64], BF16, tag="ones")
    nc.gpsimd.memset(ones_sb[:], 1.0)

    # conv1 output planes (= conv2 input planes): [128, (o,e), RP, PW]
    # zeroed once; sinks only ever write rows 1..56, cols 1..28
    yeo = [cpool.tile([128, 2, RP, PW], BF16, tag=f"yeo{c}", name=f"yeo{c}")
           for c in range(2)]
    for c in range(2):
        nc.gpsimd.memset(yeo[c][:], 0.0)

    # ---- weight generation (emitted as thunks for interleaving) ----
    def gen_weights_a_ops(wn, s, ops):
        """sigmoid(fc1(gap)) -> partition-broadcast coefficient tiles."""
        apsg = psmall_p.tile([128, 33], F32, tag="avec_ps")
        aps = apsg[:, 0:32]
        avt = small_p.tile([128, 32], F32, tag="avtmp")
        avec = small_p.tile([128, 32], BF16, tag="avec")
        avd = dram_p.tile([4096], BF16, tag="avd")
        avlin = avlin_p.tile([1, 4096], BF16, tag="avlin")
        aexp = [aexp_p.tile([128, 4 * 256], BF16, tag=f"aexp{c}",
                            name=f"aexpt{c}") for c in range(2)]
        aps2s = [psmall_p.tile([128, 2 * 256], F32, tag="aexp_ps",
                               name=f"aps2_{half}") for half in range(2)]

        ops.append(lambda: nc.sync.dma_start(fc1w_t[:], fc1wT[wn]))

        def avec_mms():
            for j in range(32):
                nc.tensor.matmul(aps[:, j:j + 1],
                                 fc1w_t[:, 128 * j:128 * (j + 1)],
                                 gap16[:, s:s + 1],
                                 start=True, stop=True)
        ops.append(avec_mms)
        ops.append(lambda: nc.vector.tensor_add(avt[:], aps, fc1b_sb[wn][:]))
        ops.append(lambda: nc.scalar.activation(avec[:], avt[:], AF.Sigmoid))
        ops.append(lambda: nc.sync.dma_start(
            avd[:].rearrange("(j p) -> p j", p=128), avec[:]))
        ops.append(lambda: nc.sync.dma_start(avlin[:], avd[:].unsqueeze(0)))
        avr = avlin[:].rearrange("o (co r) -> o co r", r=16)

        def aexp_mms(c, half):
            def f():
                aps2 = aps2s[half]
                for h in range(2):
                    for ii in range(2):
                        i = 2 * half + ii
                        m = 4 * (2 * c + h) + i
                        rhs = avr[:, :, m:m + 1].rearrange("o co r -> o (co r)")
                        nc.tensor.matmul(
                            aps2[64 * h:64 * (h + 1), 256 * ii:256 * (ii + 1)],
                            ones_sb[:], rhs, start=True, stop=True)
                nc.scalar.copy(aexp[c][:, 512 * half:512 * (half + 1)],
                               aps2[:])
            return f
        for c in range(2):
            for half in range(2):
                ops.append(aexp_mms(c, half))
        return aexp

    def gen_weights_b_ops(wn, aexp, ops):
        """combine 4 basis tensors -> W [128, 9, 256] + 1D U-transform.

        u12[:, kh, 0] = 0.5(W0+W2) + W1h   (wpos1 stationary)
        u12[:, kh, 1] = W1h - 0.5(W0+W2)   (NEGATED wpos2 stationary)
        W1h pre-halved on host.
        """
        res = []
        for c in range(2):
            t = wg_p.tile([128, 9, 256], BF16, tag=f"wg{wn}{c}",
                          name=f"wg{wn}{c}")
            u12 = wg_p.tile([128, 3, 2, 256], BF16, tag=f"u12{wn}{c}",
                            name=f"u12{wn}{c}")

            def abid(i, c=c):
                return (aexp[c][:, 256 * i:256 * (i + 1)].unsqueeze(1)
                        .broadcast_to([128, 9, 256]))

            def k3(ap2d):
                return ap2d.rearrange("p (k co) -> p k co", k=9)

            def mk(c=c, t=t, u12=u12):
                tmps = [wtmp_p.tile([128, 9, 256], BF16,
                                    tag=("wtmpP" if i == 2 else "wtmpD"),
                                    bufs=1, name=f"wtmp{i}") for i in range(3)]
                # independent Pool mul first: overlaps the DVE chain
                yield lambda: nc.gpsimd.tensor_mul(
                    tmps[2][:], k3(w2sb[wn][c][3][:]), abid(3, c))
                yield lambda: nc.vector.tensor_mul(
                    t[:], k3(w2sb[wn][c][0][:]), abid(0, c))
                for i in range(1, 3):
                    tmp = tmps[i - 1]
                    yield lambda i=i, tmp=tmp: nc.vector.tensor_mul(
                        tmp[:], k3(w2sb[wn][c][i][:]), abid(i, c))
                    yield lambda tmp=tmp: nc.vector.tensor_add(
                        t[:], t[:], tmp[:])
                yield lambda: nc.vector.tensor_add(t[:], t[:], tmps[2][:])
                for kh in range(3):
                    uu = wtmp_p.tile([128, 256], BF16, tag="utmp",
                                     name=f"uu{kh}")

                    def uops(kh=kh, uu=uu):
                        nc.vector.tensor_add(uu[:], t[:, 3 * kh, :],
                                             t[:, 3 * kh + 2, :])
                        nc.vector.tensor_scalar_mul(uu[:], uu[:], 0.5)
                        nc.vector.tensor_add(u12[:, kh, 0, :], uu[:],
                                             t[:, 3 * kh + 1, :])
                        nc.vector.tensor_sub(u12[:, kh, 1, :],
                                             t[:, 3 * kh + 1, :], uu[:])
                    yield uops
            ops.extend(mk())
            res.append((t, u12))
        return res

    def gen_weights_ops(wn, s, ops):
        return gen_weights_b_ops(wn, gen_weights_a_ops(wn, s, ops), ops)

    def stat(wu, cc, kh, wpos, occ):
        t, u12 = wu[cc]
        if wpos == 0:
            return t[:, 3 * kh + 0, 128 * occ:128 * occ + 128]
        if wpos == 3:
            return t[:, 3 * kh + 2, 128 * occ:128 * occ + 128]
        return u12[:, kh, wpos - 1, 128 * occ:128 * occ + 128]

    # ---- x loading + gap ----
    def load_x_ops(s, ops):
        planes = []
        for c in range(2):
            te = xeo_p.tile([128, 2, RP, PW], BF16, tag=f"xeo{c}",
                            name=f"xeo{c}")
            planes.append(te)
        gacc = []
        for c in range(2):
            ga = [small_p.tile([128, 1], F32, tag=f"gacc{c}{a}",
                               name=f"gacc{c}{a}") for a in range(2)]
            gs = small_p.tile([128, 1], F32, tag=f"gsum{c}", name=f"gsum{c}")
            gacc.append((ga, gs))
        apsg = psmall_p.tile([128, 33], F32, tag="avec_ps", name="gapps")
        gps = apsg[0:16, 32:33]

        for c in range(2):
            ops.append(lambda c=c: nc.sync.dma_start(planes[c][:], xeo4[s, c]))
        for c in range(2):
            for a in range(2):
                ops.append(lambda c=c, a=a: nc.scalar.activation(
                    garb[:], planes[c][:, a].rearrange("p h w -> p (h w)"),
                    AF.Copy, accum_out=gacc[c][0][a][:]))
            ops.append(lambda c=c: nc.vector.tensor_add(
                gacc[c][1][:], gacc[c][0][0][:], gacc[c][0][1][:]))

        def gapmm():
            for c in range(2):
                nc.tensor.matmul(gps, rwT_sb[c][:], gacc[c][1][:],
                                 start=(c == 0), stop=(c == 1))
            nc.scalar.activation(gap16[:, s:s + 1], gps, AF.Identity,
                                 bias=rb_sb[:], scale=1.0)
        ops.append(gapmm)
        return planes

    # ---- conv: per-t-group V strips + matmuls + sink ----
    # V planes (from o-plane ol/orr, e-plane el/er):
    #   V0 = el - er; V1 = ol + er; V2 = er - ol; V3 = ol - orr
    def conv(wu, planes, sink, bg=None, bg_rate=None):
        for t in range(NT):
            r0 = TR * t
            tws = []
            for c in range(2):
                st = twS_p.tile([128, 4, TR + 2, TC], BF16, tag=f"tws{c}",
                                name=f"tws{c}")
                e = planes[c][:, 1, r0:r0 + TR + 2, :]
                o = planes[c][:, 0, r0:r0 + TR + 2, :]
                el = e[:, :, 0:TC]
                er = e[:, :, 1:TC + 1]
                ol = o[:, :, 1:TC + 1]
                orr = o[:, :, 2:TC + 2]
                nc.vector.tensor_sub(st[:, 0], el, er)
                nc.vector.tensor_add(st[:, 1], ol, er)
                nc.gpsimd.tensor_sub(st[:, 2], er, ol)
                nc.gpsimd.tensor_sub(st[:, 3], ol, orr)
                tws.append(st)
            if bg:
                k = -(-len(bg) // (NT - t))   # ceil: finish by last group
                for _ in range(k):
                    if bg:
                        bg.popleft()()
            for occ in range(2):
                ps = psum_p.tile([128, 4, 256], F32, tag="cps")
                for wpos in range(4):
                    for cc in range(2):
                        for kh in range(3):
                            nc.tensor.matmul(
                                ps[:, wpos, 0:NG],
                                stat(wu, cc, kh, wpos, occ),
                                tws[cc][:, wpos, kh:kh + TR, :],
                                start=(cc == 0 and kh == 0),
                                stop=(cc == 1 and kh == 2))
                sink(occ, t, ps)

    # Y-stage: with M2' = -M2 (negated U2 stationary):
    #   [e1,o1] = m[(0,2)] + bcast(m1);  [e2,o2] = [e1,o1] - m[(2,3)]
    #   e2 = M0+M1+M2 (even outputs), o2 = M1-M2-M3 (odd outputs)
    def ystage(ps, t=0):
        m = stage_p.tile([128, 4, NG], BF16, tag="mev")
        nc.scalar.copy(m[:], ps[:, :, 0:NG])
        m02 = m[:].rearrange("p (j two) n -> p j two n", two=2)[:, :, 0]
        eo1 = stage_p.tile([128, 2, NG], BF16, tag="eo1", bufs=1)
        nc.vector.tensor_add(
            eo1[:], m02, m[:, 1].unsqueeze(1).broadcast_to([128, 2, NG]))
        eo2 = stage_p.tile([128, 2, NG], BF16, tag="eo2")
        nc.vector.tensor_sub(eo2[:], eo1[:], m[:, 2:4])
        return eo2

    def sink1(occ, t, ps):
        r0 = TR * t + 1
        eo2 = ystage(ps, t)
        # even outputs -> o-plane, odd outputs -> e-plane, cols 1..28
        nc.scalar.activation(
            yeo[occ][:, :, r0:r0 + TR, 1:TC + 1],
            eo2[:].rearrange("p a (h w) -> p a h w", h=TR),
            AF.Relu, bias=bnb_sb[0][occ][:], scale=1.0)

    def make_sink2(s, xplanes):
        def sink2(occ, t, ps):
            r0 = TR * t + 1
            eo2 = ystage(ps, t)
            rx = xplanes[occ][:, :, r0:r0 + TR, 1:TC + 1]
            eo3 = stage_p.tile([128, 2, TR, TC], BF16, tag="eo3")
            nc.vector.tensor_add(
                eo3[:], eo2[:].rearrange("p a (h w) -> p a h w", h=TR), rx)
            os_ = stage_p.tile([128, 2, TR, TC], BF16, tag="ostg")
            nc.scalar.activation(os_[:], eo3[:], AF.Relu,
                                 bias=bnb_sb[1][occ][:], scale=1.0)
            nc.sync.dma_start(outd[s, occ][:, :, TR * t:TR * t + TR, :],
                              os_[:])
        return sink2

    # ---- main pipeline ----
    from collections import deque
    ops0 = []
    xp = load_x_ops(0, ops0)
    for f in ops0:
        f()
    ops0 = []
    w1 = gen_weights_ops(0, 0, ops0)
    load_w2sb(0)
    for f in ops0:
        f()
    load_deferred_consts()
    load_w2sb(1)

    for s in range(BL):
        bg1 = deque()
        w2 = gen_weights_ops(1, s, bg1)
        bg2 = deque()
        if s + 1 < BL:
            xp_n = load_x_ops(s + 1, bg2)
            w1_n = gen_weights_ops(0, s + 1, bg2)

        conv(w1, xp, sink1, bg=bg1)
        while bg1:
            bg1.popleft()()
        conv(w2, yeo, make_sink2(s, xp), bg=bg2)
        while bg2:
            bg2.popleft()()

        if s + 1 < BL:
            xp, w1 = xp_n, w1_n

    ctx.close()


_NC_CACHE = {}


def get_program():
    if "nc" not in _NC_CACHE:
        _NC_CACHE["nc"] = build_program()
    return _NC_CACHE["nc"]


def prep_inputs(inputs):
    x = np.asarray(inputs["x"], np.float32)
    f32 = lambda a: np.ascontiguousarray(np.asarray(a, np.float32))
    bf = lambda a: np.ascontiguousarray(
        np.asarray(a, np.float32).astype(ml_dtypes.bfloat16))

    def bn_fold(g, b, m, v):
        sc = np.asarray(g, np.float32) / np.sqrt(np.asarray(v, np.float32) + EPS)
        bia = np.asarray(b, np.float32) - np.asarray(m, np.float32) * sc
        return sc, f32(bia.reshape(2, 128, 1))

    def pack_w2(fc2_w, bn_sc):
        w2_ = np.asarray(fc2_w, np.float32).reshape(256, 4, 64, 9, 4)
        w2_ = w2_ * bn_sc[:, None, None, None, None]   # fold bn scale (per oc)
        w2_[:, :, :, 1::3, :] *= 0.5                   # pre-halve kw=1 taps
        w2h = w2_.transpose(4, 3, 1, 2, 0).reshape(4, 9, 256, 256)
        return bf(w2h.transpose(2, 0, 1, 3).reshape(2, 128, 4 * 9 * 256))

    s1, b1 = bn_fold(inputs["bn1_g"], inputs["bn1_b"],
                     inputs["bn1_m"], inputs["bn1_v"])
    s2, b2 = bn_fold(inputs["bn2_g"], inputs["bn2_b"],
                     inputs["bn2_m"], inputs["bn2_v"])

    NPIX = H * W
    base = {
        "rwT": f32((np.asarray(inputs["reduce_w"], np.float32).T / NPIX)
                   .reshape(2, 128, 16)),
        "rb": f32(np.asarray(inputs["reduce_b"]).reshape(16, 1)),
        "fc1wT1": bf(np.asarray(inputs["w1_fc1_w"]).T),
        "fc1wT2": bf(np.asarray(inputs["w2_fc1_w"]).T),
        "fc1b1": f32(np.asarray(inputs["w1_fc1_b"]).reshape(32, 128).T),
        "fc1b2": f32(np.asarray(inputs["w2_fc1_b"]).reshape(32, 128).T),
        "w2p1": pack_w2(inputs["w1_fc2_w"], s1),
        "w2p2": pack_w2(inputs["w2_fc2_w"], s2),
        "bnb1": b1,
        "bnb2": b2,
    }

    # host-prepadded planes: o-plane[j] = xpad[2j-1] (x even cols, at 1..28),
    # e-plane[j] = xpad[2j] (x odd cols at 1..28; col 0 = xpad[0] = 0)
    xb = x.astype(ml_dtypes.bfloat16)
    xeo = np.zeros((B, C, 2, RP, PW), ml_dtypes.bfloat16)
    xeo[:, :, 0, 1:RP - 1, 1:TC + 1] = xb[:, :, :, 0::2]
    xeo[:, :, 1, 1:RP - 1, 1:TC + 1] = xb[:, :, :, 1::2]

    in_maps = []
    for i in range(NCORES):
        m = dict(base)
        m["xeo4"] = np.ascontiguousarray(
            xeo[i * BL:(i + 1) * BL].reshape(BL, 2, 128, 2, RP, PW))
        in_maps.append(m)
    return in_maps


def unpack_outputs(results):
    outs = []
    for r in results:
        od = np.asarray(r["outd"], ml_dtypes.bfloat16).astype(np.float32)
        out = np.zeros((BL, 2, 128, H, W), np.float32)
        out[..., 0::2] = od[:, :, :, 0]
        out[..., 1::2] = od[:, :, :, 1]
        outs.append(out.reshape(BL, C, H, W))
    return np.concatenate(outs, axis=0)


def kernel(**inputs):
    in_maps = prep_inputs(inputs)
    nc = get_program()
    res = bass_utils.run_bass_kernel_spmd(nc, in_maps,
                                          core_ids=list(range(NCORES)))
    return unpack_outputs(res.results)

